# revision 32
# baseline (speedup 1.0000x reference)
"""CONAD GCN forward on 8 Trainium2 NeuronCores.

Strategy
--------
Reference computes 5 GCN convs sharing one normalized adjacency A
(PyG gcn_norm with re-added self loops), then adj_ = h_ @ h_.T.

Math rewrites used here:
  * gcn_conv(x, W) = A (x W) + b = (A x) W + b  -- propagate on the
    64-wide side always, so every A-pass streams only 64 features.
  * enc2 and the two decoder heads that consume h share A @ h, so the
    whole net needs just 4 A-passes (P1..P4) + 1 pre-multiply + heads.
  * A is materialized dense on the host (0.33% nnz) and row-sharded
    over the 8 cores; each A-pass is then a plain dense matmul on the
    tensor engine, out(=P^T) = H_full^T @ A_i^T.
  * Nodes are padded 10000 -> 10240 so every tile is a full 128
    partitions (the HWDGE splits a DMA across `largest divisor of the
    partition count <= 16` SDMA engines -- 125-row tiles would get 5
    engines, 128-row tiles get all 16).
  * Per-core A^T shard is bf16 and mostly SBUF-resident across all 4
    passes; features gathered between layers with an 8-core AllGather,
    split in two so the second half overlaps the propagation of the
    first.
  * adj_ row block = h_i @ h_full^T from the gathered struct features,
    emitted bf16 and widened to f32 on the host.

Shapes: 10240 nodes = 80 tiles x 128; per core 1280 rows = 10 tiles.
"""

import sys

for _p in ("/opt/trn_rl_repo",):
    if _p not in sys.path:
        sys.path.append(_p)

import numpy as np
import ml_dtypes

import concourse.bass as bass
import concourse.mybir as mybir
import concourse.tile as tile
from concourse import bacc
from concourse import bass_utils
from concourse.masks import make_identity

BF16 = ml_dtypes.bfloat16

N = 10000                  # real nodes
IN_DIM = 256
HID = 64
NCORES = 8
P = 128                    # nodes per partition tile
NP = 10240                 # padded node count
NT = NP // P               # 80 node tiles
ROWS = NP // NCORES        # 1280 rows per core
LT = ROWS // P             # 10 local node tiles
CHUNKS = [(0, 512), (512, 512), (1024, 256)]   # tiling of the 1280 free dim
A_RES = 42                 # A^T tiles resident in SBUF; rest streamed per pass
AG_SPLIT = 5               # local tiles in the first AllGather half

F32 = mybir.dt.float32
BF = mybir.dt.bfloat16


def _build_kernel():
    nc = bacc.Bacc("TRN2", target_bir_lowering=False, debug=False,
                   num_devices=NCORES)

    # ---- I/O ---------------------------------------------------------
    at = nc.dram_tensor("at", [P, NT, ROWS], BF, kind="ExternalInput")
    xT = nc.dram_tensor("xT", [128, 2, NP], BF, kind="ExternalInput")
    w1 = nc.dram_tensor("w1", [128, 2, HID], BF, kind="ExternalInput")
    w2 = nc.dram_tensor("w2", [HID, HID], BF, kind="ExternalInput")
    aw1 = nc.dram_tensor("aw1", [HID, HID], BF, kind="ExternalInput")
    aw2 = nc.dram_tensor("aw2", [HID, IN_DIM], BF, kind="ExternalInput")
    sw = nc.dram_tensor("sw", [HID, IN_DIM], BF, kind="ExternalInput")
    b1 = nc.dram_tensor("b1", [HID, 1], F32, kind="ExternalInput")
    b2 = nc.dram_tensor("b2", [HID, 1], F32, kind="ExternalInput")
    ab1 = nc.dram_tensor("ab1", [HID, 1], F32, kind="ExternalInput")
    ab2 = nc.dram_tensor("ab2", [128, 2], F32, kind="ExternalInput")
    sb = nc.dram_tensor("sb", [128, 2], F32, kind="ExternalInput")
    x_out = nc.dram_tensor("x_out", [ROWS, IN_DIM], F32, kind="ExternalOutput")
    adj_out = nc.dram_tensor("adj_out", [ROWS, NP], BF, kind="ExternalOutput")

    rg = [list(range(NCORES))]

    with tile.TileContext(nc) as tc:
        # Round-robin issuing engine for bulk DMAs (two HWDGE rings +
        # the gpsimd SWDGE queue for three-way spreading).
        dma_rr = [nc.sync, nc.scalar]
        dma_rr3 = [nc.sync, nc.scalar, nc.gpsimd]

        def dma(i, **kw):
            dma_rr[i % len(dma_rr)].dma_start(**kw)

        def dma3(i, **kw):
            dma_rr3[i % len(dma_rr3)].dma_start(**kw)

        with (
            tc.tile_pool(name="ares", bufs=1) as ares_pool,
            tc.tile_pool(name="astr", bufs=8) as astr_pool,
            tc.tile_pool(name="xstr", bufs=4) as xstr_pool,
            tc.tile_pool(name="feat", bufs=1) as feat_pool,
            tc.tile_pool(name="small", bufs=1) as small_pool,
            tc.tile_pool(name="work", bufs=2) as work_pool,
            tc.tile_pool(name="wide", bufs=1) as wide_pool,
            tc.tile_pool(name="adjw", bufs=4) as adjw_pool,
            tc.tile_pool(name="psA", bufs=1, space="PSUM") as psA,
            tc.tile_pool(name="psB", bufs=2, space="PSUM") as psB,
            tc.tile_pool(name="psT", bufs=3, space="PSUM") as psT,
            tc.tile_pool(name="dram", bufs=1, space="DRAM") as dram,
        ):
            # ---- constants & weights ---------------------------------
            ident64 = small_pool.tile([64, 64], BF, tag="id64")
            make_identity(nc, ident64)
            ident128 = small_pool.tile([128, 128], F32, tag="id128")
            make_identity(nc, ident128)

            # w1 goes first on the gpsimd (SWDGE) queue: layer 1 needs it
            # immediately and it must not sit behind the bulk A loads.
            w1_sb = small_pool.tile([128, 2, HID], BF, tag="w1")
            nc.gpsimd.dma_start(out=w1_sb[:, :, :], in_=w1.ap()[:, :, :])

            # ---- resident A^T tiles (3-way ring round-robin) ---------
            a_res = []
            for t in range(A_RES):
                tl = ares_pool.tile([P, ROWS], BF, tag=f"a{t}")
                dma(t, out=tl[:, :], in_=at.ap()[:, t, :])
                a_res.append(tl)

            # ---- DRAM bounce buffers for collectives -----------------
            # Feature AllGather is split in two halves (by local tile)
            # so the gather of the second half overlaps the propagation
            # over the first.
            H0 = AG_SPLIT * HID
            H1 = (LT - AG_SPLIT) * HID
            ag_in = [[dram.tile([P, H0], BF, tag=f"agiA{l}",
                                name=f"ag_inA{l}"),
                      dram.tile([P, H1], BF, tag=f"agiB{l}",
                                name=f"ag_inB{l}")]
                     for l in range(4)]
            ag_out = [[dram.tile([NCORES, P, H0], BF, addr_space="Shared",
                                 tag=f"agoA{l}", name=f"ag_outA{l}"),
                       dram.tile([NCORES, P, H1], BF, addr_space="Shared",
                                 tag=f"agoB{l}", name=f"ag_outB{l}")]
                      for l in range(4)]
            # Struct-feature AllGather, split by node-column chunk so the
            # adjacency pass can start on chunk 0 before the rest lands.
            ag_in_h = [dram.tile([2, 128, w], BF, tag=f"agih{c}",
                                 name=f"ag_in_h{c}")
                       for c, (_, w) in enumerate(CHUNKS)]
            ag_out_h = [dram.tile([NCORES, 2, 128, w], BF,
                                  addr_space="Shared", tag=f"agoh{c}",
                                  name=f"ag_out_h{c}")
                        for c, (_, w) in enumerate(CHUNKS)]

            # ---- helpers ---------------------------------------------
            def propagate(feat_halves, scope):
                """P^T[64, 1280] (3 psum chunks) = feat_full^T @ A_i^T.

                feat_halves = (featA [P, 8, H0], featB [P, 8, H1]);
                iterate the A-half tiles first so the B AllGather can
                still be in flight when the pass starts.
                """
                featA, featB = feat_halves
                order = ([(r, lt) for r in range(NCORES)
                          for lt in range(AG_SPLIT)] +
                         [(r, lt) for r in range(NCORES)
                          for lt in range(AG_SPLIT, LT)])
                with nc.named_scope(scope):
                    ps = [psA.tile([HID, w], F32, tag=f"ps{c}",
                                   name=f"{scope}_ps{c}")
                          for c, (_, w) in enumerate(CHUNKS)]
                    for i, (r, lt) in enumerate(order):
                        t = r * LT + lt
                        if lt < AG_SPLIT:
                            lhsT = featA[:, r, lt * HID:(lt + 1) * HID]
                        else:
                            lhsT = featB[:, r,
                                         (lt - AG_SPLIT) * HID:
                                         (lt - AG_SPLIT + 1) * HID]
                        if t < A_RES:
                            a_t = a_res[t]
                        else:
                            a_t = astr_pool.tile([P, ROWS], BF, tag="astream")
                            dma3(t, out=a_t[:, :], in_=at.ap()[:, t, :])
                        for c, (o, w) in enumerate(CHUNKS):
                            nc.tensor.matmul(ps[c][:, :], lhsT, a_t[:, o:o + w],
                                             start=(i == 0), stop=(i == NT - 1))
                    return ps

            def transpose_and_gather(yT_sb, layer, scope):
                """yT [64,1280] bf16 -> local tiles -> AllGather (x2 halves)."""
                with nc.named_scope(scope):
                    feats = []
                    for half, (lo, hi) in enumerate([(0, AG_SPLIT),
                                                     (AG_SPLIT, LT)]):
                        hw = (hi - lo) * HID
                        stage = work_pool.tile([P, hw], BF,
                                               tag=f"tstage{half}",
                                               name=f"{scope}_stage{half}")
                        for j, lt in enumerate(range(lo, hi)):
                            pst = psT.tile([P, HID], BF, tag="pst",
                                           name=f"{scope}_pst{lt}")
                            nc.tensor.transpose(pst[:, :],
                                                yT_sb[:, lt * P:(lt + 1) * P],
                                                ident64[:, :])
                            nc.any.tensor_copy(
                                stage[:, j * HID:(j + 1) * HID], pst[:, :])
                        nc.sync.dma_start(out=ag_in[layer][half][:, :],
                                          in_=stage[:, :])
                        nc.gpsimd.collective_compute(
                            "AllGather", mybir.AluOpType.bypass,
                            replica_groups=rg,
                            ins=[ag_in[layer][half].opt()],
                            outs=[ag_out[layer][half].opt()],
                        )
                        feat_sb = feat_pool.tile([P, NCORES, hw], BF,
                                                 tag=f"feat{half}",
                                                 name=f"{scope}_feat{half}")
                        # gpsimd ring: not stuck behind the A streams
                        for r in range(NCORES):
                            nc.gpsimd.dma_start(
                                out=feat_sb[:, r, :],
                                in_=ag_out[layer][half][r, :, :])
                        feats.append(feat_sb)
                    return feats

            def copy_pT(ps, tag):
                """PSUM chunks -> one [64, 1280] bf16 SBUF tile."""
                out = work_pool.tile([HID, ROWS], BF, tag=tag, name=tag)
                for c, (o, w) in enumerate(CHUNKS):
                    nc.any.tensor_copy(out[:, o:o + w], ps[c][:, :])
                return out

            def head64(pT_sb, w_sb, bias, relu, tag, scope):
                """yT[64,1280] = act(W^T @ pT + b) in bf16."""
                with nc.named_scope(scope):
                    out = work_pool.tile([HID, ROWS], BF, tag=tag,
                                         name=f"{scope}_out")
                    for c, (o, w) in enumerate(CHUNKS):
                        ph = psB.tile([HID, w], F32, tag="ph",
                                      name=f"{scope}_ph{c}")
                        nc.tensor.matmul(ph[:, :], w_sb[:, :],
                                         pT_sb[:, o:o + w],
                                         start=True, stop=True)
                        if relu:
                            nc.scalar.activation(
                                out[:, o:o + w], ph[:, :],
                                mybir.ActivationFunctionType.Relu,
                                bias=bias[:, :])
                        else:
                            nc.any.tensor_scalar_add(out[:, o:o + w],
                                                     ph[:, :], bias[:, :])
                    return out

            def head256(pT_sb, w_sb, bias, out_dt, tag, scope):
                """[256,1280] = W^T @ pT + b as two [128,1280] tiles."""
                with nc.named_scope(scope):
                    outs = []
                    for mi in range(2):
                        out = wide_pool.tile([128, ROWS], out_dt,
                                             tag=f"{tag}{mi}",
                                             name=f"{scope}_out{mi}")
                        for c, (o, w) in enumerate(CHUNKS):
                            ph = psB.tile([128, w], F32, tag="ph",
                                          name=f"{scope}_ph{mi}{c}")
                            nc.tensor.matmul(
                                ph[:, :], w_sb[:, mi * 128:(mi + 1) * 128],
                                pT_sb[:, o:o + w], start=True, stop=True)
                            nc.any.tensor_scalar_add(out[:, o:o + w],
                                                     ph[:, :],
                                                     bias[:, mi:mi + 1])
                        outs.append(out)
                    return outs

            # ---- layer 1: f0 = x @ W1, computed for ALL nodes --------
            # Every core gets the full x, so layer 1 needs no AllGather;
            # the first collective then happens long after the kernel
            # entry CC barrier (~50us) has drained.
            H0 = AG_SPLIT * HID
            with nc.named_scope("l1_premul"):
                featA = feat_pool.tile([P, NCORES, H0], BF, tag="feat0",
                                       name="f0_featA")
                featB = feat_pool.tile([P, NCORES, H1], BF, tag="feat1",
                                       name="f0_featB")
                for g in range(NT // 4):
                    xt = xstr_pool.tile([128, 2, 512], BF, tag="xt",
                                        name=f"xt{g}")
                    nc.gpsimd.dma_start(out=xt[:, :, :],
                                        in_=xT.ap()[:, :, g * 512:
                                                    (g + 1) * 512])
                    for s in range(4):
                        t = g * 4 + s
                        r, lt = t // LT, t % LT
                        pf = psT.tile([128, HID], F32, tag="pst",
                                      name=f"f0ps{t}")
                        for k in range(2):
                            nc.tensor.matmul(pf[:, :],
                                             xt[:, k, s * 128:(s + 1) * 128],
                                             w1_sb[:, k, :],
                                             start=(k == 0), stop=(k == 1))
                        if lt < AG_SPLIT:
                            dst = featA[:, r, lt * HID:(lt + 1) * HID]
                        else:
                            dst = featB[:, r, (lt - AG_SPLIT) * HID:
                                        (lt - AG_SPLIT + 1) * HID]
                        nc.any.tensor_copy(dst, pf[:, :])
            feat = (featA, featB)

            # Remaining weights/biases: gpsimd queue, after the layer-1
            # x stream (they are needed only from the prop1 head on).
            w2_sb = small_pool.tile([HID, HID], BF, tag="w2")
            nc.gpsimd.dma_start(out=w2_sb[:, :], in_=w2.ap()[:, :])
            aw1_sb = small_pool.tile([HID, HID], BF, tag="aw1")
            nc.gpsimd.dma_start(out=aw1_sb[:, :], in_=aw1.ap()[:, :])
            aw2_sb = small_pool.tile([HID, IN_DIM], BF, tag="aw2")
            nc.gpsimd.dma_start(out=aw2_sb[:, :], in_=aw2.ap()[:, :])
            sw_sb = small_pool.tile([HID, IN_DIM], BF, tag="sw")
            nc.gpsimd.dma_start(out=sw_sb[:, :], in_=sw.ap()[:, :])

            def bias_tile(t, parts, tag):
                tl = small_pool.tile([parts, 1], F32, tag=tag)
                nc.gpsimd.dma_start(out=tl[:, :], in_=t.ap()[:parts, :])
                return tl

            b1_sb = bias_tile(b1, HID, "b1")
            b2_sb = bias_tile(b2, HID, "b2")
            ab1_sb = bias_tile(ab1, HID, "ab1")
            ab2_sb = small_pool.tile([128, 2], F32, tag="ab2")
            nc.gpsimd.dma_start(out=ab2_sb[:, :], in_=ab2.ap()[:, :])
            sb_sb = small_pool.tile([128, 2], F32, tag="sb")
            nc.gpsimd.dma_start(out=sb_sb[:, :], in_=sb.ap()[:, :])

            # ---- P1 -> h1 = relu(P1 + b1) ----------------------------
            ps1 = propagate(feat, "prop1")
            with nc.named_scope("head_h1"):
                h1T = work_pool.tile([HID, ROWS], BF, tag="yt")
                for c, (o, w) in enumerate(CHUNKS):
                    nc.scalar.activation(h1T[:, o:o + w], ps1[c][:, :],
                                         mybir.ActivationFunctionType.Relu,
                                         bias=b1_sb[:, :])
            feat = transpose_and_gather(h1T, 1, "ag_h1")

            # ---- P2 -> h = P2 @ W2 + b2 ------------------------------
            ps2 = propagate(feat, "prop2")
            p2T = copy_pT(ps2, "pt")
            hT = head64(p2T, w2_sb, b2_sb, False, "yt", "head_h")
            feat = transpose_and_gather(hT, 2, "ag_h")

            # ---- P3 -> x1 (attr) and h_ (struct) ---------------------
            ps3 = propagate(feat, "prop3")
            p3T = copy_pT(ps3, "pt")
            x1T = head64(p3T, aw1_sb, ab1_sb, True, "yt", "head_x1")
            hsT = head256(p3T, sw_sb, sb_sb, BF, "hs", "head_hs")
            feat = transpose_and_gather(x1T, 3, "ag_x1")
            with nc.named_scope("ag_hs"):
                for c, (o, w) in enumerate(CHUNKS):
                    for mi in range(2):
                        nc.sync.dma_start(out=ag_in_h[c][mi, :, :],
                                          in_=hsT[mi][:, o:o + w])
                    nc.gpsimd.collective_compute(
                        "AllGather", mybir.AluOpType.bypass,
                        replica_groups=rg,
                        ins=[ag_in_h[c].opt()], outs=[ag_out_h[c].opt()])

            # ---- P4 -> x_ = P4 @ aW2 + ab2 ---------------------------
            ps4 = propagate(feat, "prop4")
            p4T = copy_pT(ps4, "pt")
            xoT = head256(p4T, aw2_sb, ab2_sb, F32, "xo", "head_xo")

            with nc.named_scope("x_out"):
                for lt in range(LT):
                    xstage = work_pool.tile([P, IN_DIM], F32, tag="xstage",
                                            name=f"xstage{lt}")
                    for mi in range(2):
                        pst = psT.tile([P, 128], F32, tag="pst",
                                       name=f"xo_pst{lt}{mi}")
                        nc.tensor.transpose(
                            pst[:, :], xoT[mi][:, lt * P:(lt + 1) * P],
                            ident128[:, :])
                        nc.any.tensor_copy(
                            xstage[:, mi * 128:(mi + 1) * 128], pst[:, :])
                    nc.scalar.dma_start(out=x_out.ap()[lt * P:(lt + 1) * P, :],
                                        in_=xstage[:, :])

            # ---- adjacency reconstruction ----------------------------
            # Chunk-outer so chunk c only depends on the c-th slice of
            # the struct-feature AllGather.  All 10 row tiles of one
            # (chunk, rank) cell go out in a single batched DMA.
            adj_v = adj_out.ap().rearrange("(m p) c -> p m c", m=LT)
            with nc.named_scope("adj"):
                for c, (o, w) in enumerate(CHUNKS):
                    for r in range(NCORES):
                        rt = [adjw_pool.tile([128, 512], BF, tag=f"rhs{k}",
                                             name=f"adj_rhs{c}{r}{k}")
                              for k in range(2)]
                        for k in range(2):
                            dma(r + k, out=rt[k][:, :w],
                                in_=ag_out_h[c][r, k, :, :])
                        ost = adjw_pool.tile([P, LT, 512], BF, tag="ost",
                                             bufs=2, name=f"adj_ost{c}{r}")
                        for m in range(LT):
                            pa = psT.tile([P, w], F32, tag="pst",
                                          name=f"adj_ps{c}{r}{m}")
                            for k in range(2):
                                nc.tensor.matmul(
                                    pa[:, :],
                                    hsT[k][:, m * P:(m + 1) * P],
                                    rt[k][:, :w],
                                    start=(k == 0), stop=(k == 1))
                            # Split the PSUM drain across both copy
                            # engines so PSUM recycles faster than the
                            # PE refills it.
                            h = w * 5 // 8
                            nc.vector.tensor_copy(ost[:, m, :h], pa[:, :h])
                            nc.scalar.copy(ost[:, m, h:w], pa[:, h:])
                        dma(c * NCORES + r,
                            out=adj_v[:, :, r * ROWS + o:r * ROWS + o + w],
                            in_=ost[:, :, :w])

    nc.compile()
    return nc


_CACHE = {}


def _get_kernel():
    if "nc" not in _CACHE:
        _CACHE["nc"] = _build_kernel()
    return _CACHE["nc"]


def _host_prep(x, edge_index, enc_W1, enc_b1, enc_W2, enc_b2,
               attr_W1, attr_b1, attr_W2, attr_b2, struct_W, struct_b):
    """gcn_norm on the host + dense A^T shards, tiled for the device."""
    src = np.asarray(edge_index[0]).astype(np.int64)
    dst = np.asarray(edge_index[1]).astype(np.int64)
    w = (src != dst).astype(np.float32)
    deg = (np.bincount(dst, weights=w.astype(np.float64), minlength=N)
           .astype(np.float32) + 1.0)
    dinv = (1.0 / np.sqrt(deg)).astype(np.float32)
    # A^T[s, d] = sum of norm over edges s->d (+ self loops on the diag);
    # rows/cols >= N stay zero (padding).
    AT = np.zeros((NP, NP), np.float32)
    np.add.at(AT, (src, dst), dinv[src] * w * dinv[dst])
    AT[np.arange(N), np.arange(N)] += dinv * dinv

    def bf(a):
        return np.ascontiguousarray(a.astype(BF16))

    xp = np.zeros((NP, IN_DIM), np.float32)
    xp[:N] = np.asarray(x, np.float32)
    common = {
        "xT": bf(xp.T.reshape(2, 128, NP).transpose(1, 0, 2)),
        "w1": bf(np.asarray(enc_W1, np.float32).reshape(2, 128, HID)
                 .transpose(1, 0, 2)),
        "w2": bf(np.asarray(enc_W2, np.float32)),
        "aw1": bf(np.asarray(attr_W1, np.float32)),
        "aw2": bf(np.asarray(attr_W2, np.float32)),
        "sw": bf(np.asarray(struct_W, np.float32)),
        "b1": np.ascontiguousarray(np.asarray(enc_b1, np.float32)
                                   .reshape(HID, 1)),
        "b2": np.ascontiguousarray(np.asarray(enc_b2, np.float32)
                                   .reshape(HID, 1)),
        "ab1": np.ascontiguousarray(np.asarray(attr_b1, np.float32)
                                    .reshape(HID, 1)),
        "ab2": np.ascontiguousarray(np.asarray(attr_b2, np.float32)
                                    .reshape(2, 128).T),
        "sb": np.ascontiguousarray(np.asarray(struct_b, np.float32)
                                   .reshape(2, 128).T),
    }
    in_maps = []
    for i in range(NCORES):
        cols = slice(i * ROWS, (i + 1) * ROWS)
        at_i = (AT[:, cols].astype(BF16).reshape(NT, P, ROWS)
                .transpose(1, 0, 2))
        m = dict(common)
        m["at"] = np.ascontiguousarray(at_i)
        in_maps.append(m)
    return in_maps


def kernel(**inputs):
    nc = _get_kernel()
    in_maps = _host_prep(**inputs)
    res = bass_utils.run_bass_kernel_spmd(
        nc, in_maps, core_ids=list(range(NCORES)))
    x_ = np.concatenate([res.results[i]["x_out"] for i in range(NCORES)],
                        axis=0)[:N]
    adj = np.concatenate(
        [res.results[i]["adj_out"].astype(np.float32) for i in range(NCORES)],
        axis=0)[:N, :N]
    return x_, adj


# revision 38
# speedup vs baseline: 1.0888x; 1.0888x over previous
"""CONAD GCN forward on 8 Trainium2 NeuronCores.

Strategy
--------
Reference computes 5 GCN convs sharing one normalized adjacency A
(PyG gcn_norm with re-added self loops), then adj_ = h_ @ h_.T.

Math rewrites used here:
  * gcn_conv(x, W) = A (x W) + b = (A x) W + b  -- propagate on the
    64-wide side always, so every A-pass streams only 64 features.
  * enc2 and the two decoder heads that consume h share A @ h, so the
    whole net needs just 4 A-passes (P1..P4) + 1 pre-multiply + heads.
  * A is materialized dense on the host (0.33% nnz) and row-sharded
    over the 8 cores; each A-pass is then a plain dense matmul on the
    tensor engine, out(=P^T) = H_full^T @ A_i^T.
  * Nodes are padded 10000 -> 10240 so every tile is a full 128
    partitions (the HWDGE splits a DMA across `largest divisor of the
    partition count <= 16` SDMA engines -- 125-row tiles would get 5
    engines, 128-row tiles get all 16).
  * Per-core A^T shard is bf16 and mostly SBUF-resident across all 4
    passes; features gathered between layers with an 8-core AllGather,
    split in two so the second half overlaps the propagation of the
    first.
  * adj_ row block = h_i @ h_full^T from the gathered struct features,
    emitted bf16 and widened to f32 on the host.

Shapes: 10240 nodes = 80 tiles x 128; per core 1280 rows = 10 tiles.
"""

import sys

for _p in ("/opt/trn_rl_repo",):
    if _p not in sys.path:
        sys.path.append(_p)

import numpy as np
import ml_dtypes

import concourse.bass as bass
import concourse.mybir as mybir
import concourse.tile as tile
from concourse import bacc
from concourse import bass_utils
from concourse.masks import make_identity

BF16 = ml_dtypes.bfloat16

N = 10000                  # real nodes
IN_DIM = 256
HID = 64
NCORES = 8
P = 128                    # nodes per partition tile
NP = 10240                 # padded node count
NT = NP // P               # 80 node tiles
ROWS = NP // NCORES        # 1280 rows per core
LT = ROWS // P             # 10 local node tiles
CHUNKS = [(0, 512), (512, 512), (1024, 256)]   # tiling of the 1280 free dim
A_RES = 42                 # A^T tiles resident in SBUF; rest streamed per pass
AG_SPLIT = 5               # local tiles in the first AllGather half

F32 = mybir.dt.float32
BF = mybir.dt.bfloat16


def _build_kernel():
    nc = bacc.Bacc("TRN2", target_bir_lowering=False, debug=False,
                   num_devices=NCORES)

    # ---- I/O ---------------------------------------------------------
    at = nc.dram_tensor("at", [P, NT, ROWS], BF, kind="ExternalInput")
    xT = nc.dram_tensor("xT", [128, 2, NP], BF, kind="ExternalInput")
    w1 = nc.dram_tensor("w1", [128, 2, HID], BF, kind="ExternalInput")
    w2 = nc.dram_tensor("w2", [HID, HID], BF, kind="ExternalInput")
    aw1 = nc.dram_tensor("aw1", [HID, HID], BF, kind="ExternalInput")
    aw2 = nc.dram_tensor("aw2", [HID, IN_DIM], BF, kind="ExternalInput")
    sw = nc.dram_tensor("sw", [HID, IN_DIM], BF, kind="ExternalInput")
    b1 = nc.dram_tensor("b1", [HID, 1], F32, kind="ExternalInput")
    b2 = nc.dram_tensor("b2", [HID, 1], F32, kind="ExternalInput")
    ab1 = nc.dram_tensor("ab1", [HID, 1], F32, kind="ExternalInput")
    ab2 = nc.dram_tensor("ab2", [128, 2], F32, kind="ExternalInput")
    sb = nc.dram_tensor("sb", [128, 2], F32, kind="ExternalInput")
    x_out = nc.dram_tensor("x_out", [ROWS, IN_DIM], F32, kind="ExternalOutput")
    adj_out = nc.dram_tensor("adj_out", [ROWS, NP], BF, kind="ExternalOutput")

    rg = [list(range(NCORES))]

    with tile.TileContext(nc) as tc:
        # Round-robin issuing engine for bulk DMAs (two HWDGE rings +
        # the gpsimd SWDGE queue for three-way spreading).
        dma_rr = [nc.sync, nc.scalar]
        dma_rr3 = [nc.sync, nc.scalar, nc.gpsimd]

        def dma(i, **kw):
            dma_rr[i % len(dma_rr)].dma_start(**kw)

        def dma3(i, **kw):
            dma_rr3[i % len(dma_rr3)].dma_start(**kw)

        with (
            tc.tile_pool(name="ares", bufs=1) as ares_pool,
            tc.tile_pool(name="astr", bufs=8) as astr_pool,
            tc.tile_pool(name="xstr", bufs=4) as xstr_pool,
            tc.tile_pool(name="feat", bufs=1) as feat_pool,
            tc.tile_pool(name="small", bufs=1) as small_pool,
            tc.tile_pool(name="work", bufs=2) as work_pool,
            tc.tile_pool(name="wide", bufs=1) as wide_pool,
            tc.tile_pool(name="adjw", bufs=4) as adjw_pool,
            tc.tile_pool(name="psA", bufs=1, space="PSUM") as psA,
            tc.tile_pool(name="psB", bufs=2, space="PSUM") as psB,
            tc.tile_pool(name="psT", bufs=3, space="PSUM") as psT,
            tc.tile_pool(name="dram", bufs=1, space="DRAM") as dram,
        ):
            # ---- constants & weights ---------------------------------
            ident64 = small_pool.tile([64, 64], BF, tag="id64")
            make_identity(nc, ident64)
            ident128 = small_pool.tile([128, 128], F32, tag="id128")
            make_identity(nc, ident128)

            # w1 goes first on the gpsimd (SWDGE) queue: layer 1 needs it
            # immediately and it must not sit behind the bulk A loads.
            w1_sb = small_pool.tile([128, 2, HID], BF, tag="w1")
            nc.gpsimd.dma_start(out=w1_sb[:, :, :], in_=w1.ap()[:, :, :])



            # ---- DRAM bounce buffers for collectives -----------------
            # Feature AllGather is split in two halves (by local tile)
            # so the gather of the second half overlaps the propagation
            # over the first.
            H0 = AG_SPLIT * HID
            H1 = (LT - AG_SPLIT) * HID
            ag_in = [[dram.tile([P, H0], BF, tag=f"agiA{l}",
                                name=f"ag_inA{l}"),
                      dram.tile([P, H1], BF, tag=f"agiB{l}",
                                name=f"ag_inB{l}")]
                     for l in range(4)]
            ag_out = [[dram.tile([NCORES, P, H0], BF, addr_space="Shared",
                                 tag=f"agoA{l}", name=f"ag_outA{l}"),
                       dram.tile([NCORES, P, H1], BF, addr_space="Shared",
                                 tag=f"agoB{l}", name=f"ag_outB{l}")]
                      for l in range(4)]
            # Struct-feature AllGather, split by node-column chunk so the
            # adjacency pass can start on chunk 0 before the rest lands.
            ag_in_h = [dram.tile([2, 128, w], BF, tag=f"agih{c}",
                                 name=f"ag_in_h{c}")
                       for c, (_, w) in enumerate(CHUNKS)]
            ag_out_h = [dram.tile([NCORES, 2, 128, w], BF,
                                  addr_space="Shared", tag=f"agoh{c}",
                                  name=f"ag_out_h{c}")
                        for c, (_, w) in enumerate(CHUNKS)]

            # ---- helpers ---------------------------------------------
            def propagate(feat_halves, scope):
                """P^T[64, 1280] (3 psum chunks) = feat_full^T @ A_i^T.

                feat_halves = (featA [P, 8, H0], featB [P, 8, H1]);
                iterate the A-half tiles first so the B AllGather can
                still be in flight when the pass starts.
                """
                featA, featB = feat_halves
                order = ([(r, lt) for r in range(NCORES)
                          for lt in range(AG_SPLIT)] +
                         [(r, lt) for r in range(NCORES)
                          for lt in range(AG_SPLIT, LT)])
                with nc.named_scope(scope):
                    ps = [psA.tile([HID, w], F32, tag=f"ps{c}",
                                   name=f"{scope}_ps{c}")
                          for c, (_, w) in enumerate(CHUNKS)]
                    for i, (r, lt) in enumerate(order):
                        t = r * LT + lt
                        if lt < AG_SPLIT:
                            lhsT = featA[:, r, lt * HID:(lt + 1) * HID]
                        else:
                            lhsT = featB[:, r,
                                         (lt - AG_SPLIT) * HID:
                                         (lt - AG_SPLIT + 1) * HID]
                        if t < A_RES:
                            a_t = a_res[t]
                        else:
                            a_t = astr_pool.tile([P, ROWS], BF, tag="astream")
                            dma(t, out=a_t[:, :], in_=at.ap()[:, t, :])
                        for c, (o, w) in enumerate(CHUNKS):
                            nc.tensor.matmul(ps[c][:, :], lhsT, a_t[:, o:o + w],
                                             start=(i == 0), stop=(i == NT - 1))
                    return ps

            def transpose_and_gather(yT_sb, layer, scope):
                """yT [64,1280] bf16 -> local tiles -> AllGather (x2 halves)."""
                with nc.named_scope(scope):
                    feats = []
                    for half, (lo, hi) in enumerate([(0, AG_SPLIT),
                                                     (AG_SPLIT, LT)]):
                        hw = (hi - lo) * HID
                        stage = work_pool.tile([P, hw], BF,
                                               tag=f"tstage{half}",
                                               name=f"{scope}_stage{half}")
                        for j, lt in enumerate(range(lo, hi)):
                            pst = psT.tile([P, HID], BF, tag="pst",
                                           name=f"{scope}_pst{lt}")
                            nc.tensor.transpose(pst[:, :],
                                                yT_sb[:, lt * P:(lt + 1) * P],
                                                ident64[:, :])
                            nc.any.tensor_copy(
                                stage[:, j * HID:(j + 1) * HID], pst[:, :])
                        nc.sync.dma_start(out=ag_in[layer][half][:, :],
                                          in_=stage[:, :])
                        nc.gpsimd.collective_compute(
                            "AllGather", mybir.AluOpType.bypass,
                            replica_groups=rg,
                            ins=[ag_in[layer][half].opt()],
                            outs=[ag_out[layer][half].opt()],
                        )
                        feat_sb = feat_pool.tile([P, NCORES, hw], BF,
                                                 tag=f"feat{half}",
                                                 name=f"{scope}_feat{half}")
                        for r in range(NCORES):
                            dma(r, out=feat_sb[:, r, :],
                                in_=ag_out[layer][half][r, :, :])
                        feats.append(feat_sb)
                    return feats

            def copy_pT(ps, tag):
                """PSUM chunks -> one [64, 1280] bf16 SBUF tile."""
                out = work_pool.tile([HID, ROWS], BF, tag=tag, name=tag)
                for c, (o, w) in enumerate(CHUNKS):
                    nc.any.tensor_copy(out[:, o:o + w], ps[c][:, :])
                return out

            def head64(pT_sb, w_sb, bias, relu, tag, scope):
                """yT[64,1280] = act(W^T @ pT + b) in bf16."""
                with nc.named_scope(scope):
                    out = work_pool.tile([HID, ROWS], BF, tag=tag,
                                         name=f"{scope}_out")
                    for c, (o, w) in enumerate(CHUNKS):
                        ph = psB.tile([HID, w], F32, tag="ph",
                                      name=f"{scope}_ph{c}")
                        nc.tensor.matmul(ph[:, :], w_sb[:, :],
                                         pT_sb[:, o:o + w],
                                         start=True, stop=True)
                        if relu:
                            nc.scalar.activation(
                                out[:, o:o + w], ph[:, :],
                                mybir.ActivationFunctionType.Relu,
                                bias=bias[:, :])
                        else:
                            nc.any.tensor_scalar_add(out[:, o:o + w],
                                                     ph[:, :], bias[:, :])
                    return out

            def head256(pT_sb, w_sb, bias, out_dt, tag, scope):
                """[256,1280] = W^T @ pT + b as two [128,1280] tiles."""
                with nc.named_scope(scope):
                    outs = []
                    for mi in range(2):
                        out = wide_pool.tile([128, ROWS], out_dt,
                                             tag=f"{tag}{mi}",
                                             name=f"{scope}_out{mi}")
                        for c, (o, w) in enumerate(CHUNKS):
                            ph = psB.tile([128, w], F32, tag="ph",
                                          name=f"{scope}_ph{mi}{c}")
                            nc.tensor.matmul(
                                ph[:, :], w_sb[:, mi * 128:(mi + 1) * 128],
                                pT_sb[:, o:o + w], start=True, stop=True)
                            nc.any.tensor_scalar_add(out[:, o:o + w],
                                                     ph[:, :],
                                                     bias[:, mi:mi + 1])
                        outs.append(out)
                    return outs

            # ---- layer 1: f0 = x @ W1, computed for ALL nodes --------
            # Every core gets the full x, so layer 1 needs no AllGather;
            # the first collective then happens long after the kernel
            # entry CC barrier (~50us) has drained.
            H0 = AG_SPLIT * HID
            with nc.named_scope("l1_premul"):
                featA = feat_pool.tile([P, NCORES, H0], BF, tag="feat0",
                                       name="f0_featA")
                featB = feat_pool.tile([P, NCORES, H1], BF, tag="feat1",
                                       name="f0_featB")
                for g in range(NT // 4):
                    xt = xstr_pool.tile([128, 2, 512], BF, tag="xt",
                                        name=f"xt{g}")
                    nc.sync.dma_start(out=xt[:, :, :],
                                      in_=xT.ap()[:, :, g * 512:
                                                  (g + 1) * 512])
                    for s in range(4):
                        t = g * 4 + s
                        r, lt = t // LT, t % LT
                        pf = psT.tile([128, HID], F32, tag="pst",
                                      name=f"f0ps{t}")
                        for k in range(2):
                            nc.tensor.matmul(pf[:, :],
                                             xt[:, k, s * 128:(s + 1) * 128],
                                             w1_sb[:, k, :],
                                             start=(k == 0), stop=(k == 1))
                        if lt < AG_SPLIT:
                            dst = featA[:, r, lt * HID:(lt + 1) * HID]
                        else:
                            dst = featB[:, r, (lt - AG_SPLIT) * HID:
                                        (lt - AG_SPLIT + 1) * HID]
                        nc.any.tensor_copy(dst, pf[:, :])
            feat = (featA, featB)

            # ---- resident A^T tiles (after the layer-1 x stream in
            # program order so xt is not stuck behind them on sync) ----
            a_res = []
            for t in range(A_RES):
                tl = ares_pool.tile([P, ROWS], BF, tag=f"a{t}",
                                    name=f"ares{t}")
                dma(t, out=tl[:, :], in_=at.ap()[:, t, :])
                a_res.append(tl)

            # Remaining weights/biases: gpsimd queue, after the layer-1
            # x stream (they are needed only from the prop1 head on).
            w2_sb = small_pool.tile([HID, HID], BF, tag="w2")
            nc.gpsimd.dma_start(out=w2_sb[:, :], in_=w2.ap()[:, :])
            aw1_sb = small_pool.tile([HID, HID], BF, tag="aw1")
            nc.gpsimd.dma_start(out=aw1_sb[:, :], in_=aw1.ap()[:, :])
            aw2_sb = small_pool.tile([HID, IN_DIM], BF, tag="aw2")
            nc.gpsimd.dma_start(out=aw2_sb[:, :], in_=aw2.ap()[:, :])
            sw_sb = small_pool.tile([HID, IN_DIM], BF, tag="sw")
            nc.gpsimd.dma_start(out=sw_sb[:, :], in_=sw.ap()[:, :])

            def bias_tile(t, parts, tag):
                tl = small_pool.tile([parts, 1], F32, tag=tag)
                nc.gpsimd.dma_start(out=tl[:, :], in_=t.ap()[:parts, :])
                return tl

            b1_sb = bias_tile(b1, HID, "b1")
            b2_sb = bias_tile(b2, HID, "b2")
            ab1_sb = bias_tile(ab1, HID, "ab1")
            ab2_sb = small_pool.tile([128, 2], F32, tag="ab2")
            nc.gpsimd.dma_start(out=ab2_sb[:, :], in_=ab2.ap()[:, :])
            sb_sb = small_pool.tile([128, 2], F32, tag="sb")
            nc.gpsimd.dma_start(out=sb_sb[:, :], in_=sb.ap()[:, :])

            # ---- P1 -> h1 = relu(P1 + b1) ----------------------------
            ps1 = propagate(feat, "prop1")
            with nc.named_scope("head_h1"):
                h1T = work_pool.tile([HID, ROWS], BF, tag="yt")
                for c, (o, w) in enumerate(CHUNKS):
                    nc.scalar.activation(h1T[:, o:o + w], ps1[c][:, :],
                                         mybir.ActivationFunctionType.Relu,
                                         bias=b1_sb[:, :])
            feat = transpose_and_gather(h1T, 1, "ag_h1")

            # ---- P2 -> h = P2 @ W2 + b2 ------------------------------
            ps2 = propagate(feat, "prop2")
            p2T = copy_pT(ps2, "pt")
            hT = head64(p2T, w2_sb, b2_sb, False, "yt", "head_h")
            feat = transpose_and_gather(hT, 2, "ag_h")

            # ---- P3 -> x1 (attr) and h_ (struct) ---------------------
            ps3 = propagate(feat, "prop3")
            p3T = copy_pT(ps3, "pt")
            x1T = head64(p3T, aw1_sb, ab1_sb, True, "yt", "head_x1")
            hsT = head256(p3T, sw_sb, sb_sb, BF, "hs", "head_hs")
            feat = transpose_and_gather(x1T, 3, "ag_x1")
            with nc.named_scope("ag_hs"):
                for c, (o, w) in enumerate(CHUNKS):
                    for mi in range(2):
                        nc.sync.dma_start(out=ag_in_h[c][mi, :, :],
                                          in_=hsT[mi][:, o:o + w])
                    nc.gpsimd.collective_compute(
                        "AllGather", mybir.AluOpType.bypass,
                        replica_groups=rg,
                        ins=[ag_in_h[c].opt()], outs=[ag_out_h[c].opt()])

            # ---- P4 -> x_ = P4 @ aW2 + ab2 ---------------------------
            ps4 = propagate(feat, "prop4")
            p4T = copy_pT(ps4, "pt")
            xoT = head256(p4T, aw2_sb, ab2_sb, F32, "xo", "head_xo")

            with nc.named_scope("x_out"):
                for lt in range(LT):
                    xstage = work_pool.tile([P, IN_DIM], F32, tag="xstage",
                                            name=f"xstage{lt}")
                    for mi in range(2):
                        pst = psT.tile([P, 128], F32, tag="pst",
                                       name=f"xo_pst{lt}{mi}")
                        nc.tensor.transpose(
                            pst[:, :], xoT[mi][:, lt * P:(lt + 1) * P],
                            ident128[:, :])
                        nc.any.tensor_copy(
                            xstage[:, mi * 128:(mi + 1) * 128], pst[:, :])
                    nc.scalar.dma_start(out=x_out.ap()[lt * P:(lt + 1) * P, :],
                                        in_=xstage[:, :])

            # ---- adjacency reconstruction ----------------------------
            # Chunk-outer so chunk c only depends on the c-th slice of
            # the struct-feature AllGather.  All 10 row tiles of one
            # (chunk, rank) cell go out in a single batched DMA.
            adj_v = adj_out.ap().rearrange("(m p) c -> p m c", m=LT)
            with nc.named_scope("adj"):
                for c, (o, w) in enumerate(CHUNKS):
                    for r in range(NCORES):
                        rt = [adjw_pool.tile([128, 512], BF, tag=f"rhs{k}",
                                             name=f"adj_rhs{c}{r}{k}")
                              for k in range(2)]
                        for k in range(2):
                            dma(r + k, out=rt[k][:, :w],
                                in_=ag_out_h[c][r, k, :, :])
                        ost = adjw_pool.tile([P, LT, 512], BF, tag="ost",
                                             bufs=2, name=f"adj_ost{c}{r}")
                        for m in range(LT):
                            pa = psT.tile([P, w], F32, tag="pst",
                                          name=f"adj_ps{c}{r}{m}")
                            for k in range(2):
                                nc.tensor.matmul(
                                    pa[:, :],
                                    hsT[k][:, m * P:(m + 1) * P],
                                    rt[k][:, :w],
                                    start=(k == 0), stop=(k == 1))
                            # Alternate whole copies over both engines
                            # (2:1 -- ACT has high per-op overhead) so
                            # PSUM recycles as fast as the PE fills it.
                            if m % 3 == 2:
                                nc.scalar.copy(ost[:, m, :w], pa[:, :])
                            else:
                                nc.vector.tensor_copy(ost[:, m, :w],
                                                      pa[:, :])
                        dma(c * NCORES + r,
                            out=adj_v[:, :, r * ROWS + o:r * ROWS + o + w],
                            in_=ost[:, :, :w])

    nc.compile()
    return nc


_CACHE = {}


def _get_kernel():
    if "nc" not in _CACHE:
        _CACHE["nc"] = _build_kernel()
    return _CACHE["nc"]


def _host_prep(x, edge_index, enc_W1, enc_b1, enc_W2, enc_b2,
               attr_W1, attr_b1, attr_W2, attr_b2, struct_W, struct_b):
    """gcn_norm on the host + dense A^T shards, tiled for the device."""
    src = np.asarray(edge_index[0]).astype(np.int64)
    dst = np.asarray(edge_index[1]).astype(np.int64)
    w = (src != dst).astype(np.float32)
    deg = (np.bincount(dst, weights=w.astype(np.float64), minlength=N)
           .astype(np.float32) + 1.0)
    dinv = (1.0 / np.sqrt(deg)).astype(np.float32)
    # A^T[s, d] = sum of norm over edges s->d (+ self loops on the diag);
    # rows/cols >= N stay zero (padding).
    AT = np.zeros((NP, NP), np.float32)
    np.add.at(AT, (src, dst), dinv[src] * w * dinv[dst])
    AT[np.arange(N), np.arange(N)] += dinv * dinv

    def bf(a):
        return np.ascontiguousarray(a.astype(BF16))

    xp = np.zeros((NP, IN_DIM), np.float32)
    xp[:N] = np.asarray(x, np.float32)
    common = {
        "xT": bf(xp.T.reshape(2, 128, NP).transpose(1, 0, 2)),
        "w1": bf(np.asarray(enc_W1, np.float32).reshape(2, 128, HID)
                 .transpose(1, 0, 2)),
        "w2": bf(np.asarray(enc_W2, np.float32)),
        "aw1": bf(np.asarray(attr_W1, np.float32)),
        "aw2": bf(np.asarray(attr_W2, np.float32)),
        "sw": bf(np.asarray(struct_W, np.float32)),
        "b1": np.ascontiguousarray(np.asarray(enc_b1, np.float32)
                                   .reshape(HID, 1)),
        "b2": np.ascontiguousarray(np.asarray(enc_b2, np.float32)
                                   .reshape(HID, 1)),
        "ab1": np.ascontiguousarray(np.asarray(attr_b1, np.float32)
                                    .reshape(HID, 1)),
        "ab2": np.ascontiguousarray(np.asarray(attr_b2, np.float32)
                                    .reshape(2, 128).T),
        "sb": np.ascontiguousarray(np.asarray(struct_b, np.float32)
                                   .reshape(2, 128).T),
    }
    in_maps = []
    for i in range(NCORES):
        cols = slice(i * ROWS, (i + 1) * ROWS)
        at_i = (AT[:, cols].astype(BF16).reshape(NT, P, ROWS)
                .transpose(1, 0, 2))
        m = dict(common)
        m["at"] = np.ascontiguousarray(at_i)
        in_maps.append(m)
    return in_maps


def kernel(**inputs):
    nc = _get_kernel()
    in_maps = _host_prep(**inputs)
    res = bass_utils.run_bass_kernel_spmd(
        nc, in_maps, core_ids=list(range(NCORES)))
    x_ = np.concatenate([res.results[i]["x_out"] for i in range(NCORES)],
                        axis=0)[:N]
    adj = np.concatenate(
        [res.results[i]["adj_out"].astype(np.float32) for i in range(NCORES)],
        axis=0)[:N, :N]
    return x_, adj
